# revision 2
# baseline (speedup 1.0000x reference)
"""Causal self-attention (B=4, T=2048, C=1024, 16 heads, interleaved RoPE)
on 8 trn2 NeuronCores.

Sharding: hybrid batch x head-half. Core c owns batch c//2 and heads
8*(c%2)..8*(c%2)+7 (512 channels). Host sums the two partial [T, C]
outputs per batch (the all-reduce of the hinted TP scheme).

Per-core data path is fp16 (f32 PSUM accumulation), which keeps every
matmul at 1 PE-cycle/row and enables:
  - x^T tiles loaded straight from DRAM with the DMA XBAR transpose
    (zero PE transposes for x),
  - interleaved RoPE without PE matmuls: head dims are pre-permuted
    host-side so the even/odd pair swap becomes a 16-lane swap within
    32-lane groups, done by one DVE stream_shuffle,
  - V projected token-major directly (x^T chunks stationary), so V^T
    needs no transposes either,
  - exact block-causal score/PV ranges (fp16 has no N>=256 rate cliff).
Scores S^T[kv, q] = K Q^T per head in double-wide [128, 1024] PSUM
tiles; exp on ACT (scale=1/8, no max subtraction; scores are ~N(0,1));
causal via column subranges + one triangular mask multiply per diagonal
block. y^T = V_aug^T @ P^T with a fused ones-column row-sum;
normalization via DMA lane-spread reciprocal + gpsimd broadcast.
Score->exp->PV is software-pipelined (S of pair p+1 issues before PV of
pair p) and stage-1/out-proj pieces are interleaved between attention
heads to keep the PE stream dense.

Self-contained: hardcoded shapes, no reads of /root/problem/*.
"""
import numpy as np

import concourse.bacc as bacc
import concourse.mybir as mybir
import concourse.tile as tile
from concourse.bass_utils import run_bass_kernel_spmd
from concourse.masks import make_upper_triangular

B, T, C = 4, 2048, 1024
NH, D = 16, 64
NCORES = 8
HPC = 8  # heads per core
HD = HPC * D  # per-core head channels = 512
CB = C // 128  # input channel blocks = 8
HB = HD // 128  # head-channel blocks = 4
QT = 512
NJ = T // QT  # q tiles = 4
KB = T // 128  # kv blocks = 16
F16 = mybir.dt.float16
F32 = mybir.dt.float32
EXP = mybir.ActivationFunctionType.Exp
SWAP16 = list(range(16, 32)) + list(range(16))

_CACHE = {}


def build():
    nc = bacc.Bacc(None, target_bir_lowering=False)
    x_d = nc.declare_dram_parameter("x", [T, C], F16, isOutput=False)
    wq_d = nc.declare_dram_parameter("wqt", [C, HD], F16, isOutput=False)
    wk_d = nc.declare_dram_parameter("wkt", [C, HD], F16, isOutput=False)
    wv_d = nc.declare_dram_parameter("wvt", [C, HD], F16, isOutput=False)
    wo_d = nc.declare_dram_parameter("wot", [HD, C], F16, isOutput=False)
    cos_d = nc.declare_dram_parameter("cosb", [128, T], F32, isOutput=False)
    sin_d = nc.declare_dram_parameter("sinb", [128, T], F32, isOutput=False)
    out_d = nc.declare_dram_parameter("out", [T, C], F32, isOutput=True)

    with tile.TileContext(nc) as tc:
        with (
            tc.tile_pool(name="const", bufs=1) as const,
            tc.tile_pool(name="wpool", bufs=1) as wpool,
            tc.tile_pool(name="xtp", bufs=2) as xtp,
            tc.tile_pool(name="rope", bufs=2) as rope,
            tc.tile_pool(name="qkp", bufs=1) as qkp,
            tc.tile_pool(name="ptp", bufs=4) as ptp,
            tc.tile_pool(name="yup", bufs=4) as yup,
            tc.tile_pool(name="npool", bufs=4) as npool,
            tc.tile_pool(name="opool", bufs=4) as opool,
            tc.tile_pool(name="ps", bufs=2, space="PSUM") as ps,
        ):
            # ---- constants ----
            cos_t = const.tile([128, T], F32)
            sin_t = const.tile([128, T], F32)
            tri = const.tile([128, 128], F16)
            ones16 = const.tile([128, KB * HPC], F16)
            with tc.tile_pool(name="wstage", bufs=1) as wstage:
                tri_f = wstage.tile([128, 128], F32, tag="trif")
                make_upper_triangular(nc, tri_f, val=1.0, diag=True)  # 1 if i<=j
                nc.vector.tensor_copy(tri[:], tri_f[:])
                ones_f = wstage.tile([128, KB * HPC], F32, tag="onesf")
                nc.gpsimd.memset(ones_f[:], 1.0)
                nc.vector.tensor_copy(ones16[:], ones_f[:])

            # ---- weights (fp16, no casting needed) ----
            wq_sb = wpool.tile([128, CB, HD], F16)
            wk_sb = wpool.tile([128, CB, HD], F16)
            wv_sb = wpool.tile([128, CB, HD], F16)
            wo_sb = wpool.tile([128, HB, C], F16)

            def load_weights():
                # spread across the three DMA-capable queues so nothing
                # serializes behind the startup x^T transposes
                nc.gpsimd.dma_start(
                    out=wq_sb[:], in_=wq_d.ap().rearrange("(cb p) m -> p cb m", p=128)
                )
                nc.gpsimd.dma_start(
                    out=wv_sb[:], in_=wv_d.ap().rearrange("(cb p) m -> p cb m", p=128)
                )
                nc.gpsimd.dma_start(out=sin_t[:], in_=sin_d[:])
                nc.scalar.dma_start(
                    out=wk_sb[:], in_=wk_d.ap().rearrange("(cb p) m -> p cb m", p=128)
                )
                nc.sync.dma_start(out=cos_t[:], in_=cos_d[:])
                nc.scalar.dma_start(
                    out=wo_sb[:], in_=wo_d.ap().rearrange("(hb p) m -> p hb m", p=128)
                )

            # ---- persistent per-batch tensors ----
            # K^T stored twice, zero-padded to K=128 per head parity: even
            # heads live in rows 0-63 of ktE (rows 64-127 zero), odd heads in
            # rows 64-127 of ktO. Scores then run with K=128 (full PE rate;
            # K=64 matmuls measure at half rate), the garbage rows of the
            # moving Q operand multiply zeros.
            qt = qkp.tile([128, HB, T], F16)
            ktE = qkp.tile([128, HB, T], F16)
            ktO = qkp.tile([128, HB, T], F16)
            yt = qkp.tile([128, HB, T], F16)
            va = qkp.tile([128, KB, HPC, D + 1], F16)
            nc.vector.memset(ktE[D:128, :, :], 0.0)
            nc.vector.memset(ktO[0:D, :, :], 0.0)
            nc.vector.tensor_copy(
                va[:, :, :, D : D + 1], ones16[:].rearrange("p (k h) -> p k h", h=HPC)
            )

            xts = {}

            def stage1_pieces(tt):
                """Closures: transpose-load + project + rope 512 tokens."""
                t0 = tt * QT
                pieces = []

                def load_xt():
                    xts[tt] = xtp.tile(
                        [128, CB, QT], F16, name=f"xt_{tt}", tag="xt", bufs=3
                    )
                    for cb in range(CB):
                        # at startup scalar is idle: split transposes across
                        # both hwdge queues to halve time-to-first-matmul
                        eng = nc.scalar if (tt == 0 and cb % 2) else nc.sync
                        eng.dma_start(
                            out=xts[tt][:, cb],
                            in_=x_d.ap()[t0 : t0 + QT, cb * 128 : (cb + 1) * 128],
                            transpose=True,
                        )

                pieces.append(load_xt)

                def qk_piece(wsb, dst, hb):
                    xt = xts[tt]
                    pj = ps.tile([128, QT], F32, name="pj", tag="pj", bufs=2)
                    for cb in range(CB):
                        nc.tensor.matmul(
                            pj[:], wsb[:, cb, hb * 128 : (hb + 1) * 128], xt[:, cb],
                            start=(cb == 0), stop=(cb == CB - 1),
                        )
                    pjc = rope.tile([128, QT], F16, name="pjc", tag="pjc", bufs=2)
                    nc.vector.tensor_mul(pjc[:], pj[:], cos_t[:, t0 : t0 + QT])
                    pjs = rope.tile([128, QT], F16, name="pjs", tag="pjs", bufs=2)
                    nc.vector.tensor_mul(pjs[:], pj[:], sin_t[:, t0 : t0 + QT])
                    pjss = rope.tile([128, QT], F16, name="pjss", tag="pjss", bufs=2)
                    nc.vector.stream_shuffle(pjss[:], pjs[:], SWAP16)
                    if dst is qt:
                        nc.vector.tensor_add(dst[:, hb, t0 : t0 + QT], pjc[:], pjss[:])
                    else:
                        nc.vector.tensor_add(
                            ktE[0:D, hb, t0 : t0 + QT], pjc[0:D], pjss[0:D]
                        )
                        nc.vector.tensor_add(
                            ktO[D:128, hb, t0 : t0 + QT], pjc[D:128], pjss[D:128]
                        )

                for wsb, dst in ((wq_sb, qt), (wk_sb, None)):
                    for hb in range(HB):
                        pieces.append(
                            lambda wsb=wsb, dst=dst, hb=hb: qk_piece(wsb, dst, hb)
                        )

                def v_piece(tb):
                    xt = xts[tt]
                    pj = ps.tile([128, QT], F32, name="pjv", tag="pj", bufs=2)
                    for cb in range(CB):
                        nc.tensor.matmul(
                            pj[:], xt[:, cb, tb * 128 : (tb + 1) * 128], wv_sb[:, cb],
                            start=(cb == 0), stop=(cb == CB - 1),
                        )
                    kv = tt * 4 + tb
                    nc.vector.tensor_copy(
                        va[:, kv, :, 0:D], pj[:].rearrange("p (h d) -> p h d", d=D)
                    )

                for tb in range(4):
                    pieces.append(lambda tb=tb: v_piece(tb))
                return pieces

            def attention(j, h):
                hr = (h % 2) * D
                hb = h // 2
                ktX = ktO if (h % 2) else ktE
                q0 = j * QT
                nblk = 4 * (j + 1)
                ytps = ps.tile([D + 1, QT], F32, name="ytps", tag="yt", bufs=2)
                pend = None

                def do_pv(halves, pt):
                    for idx, k, e0 in halves:
                        nc.tensor.matmul(
                            ytps[:, e0:QT],
                            va[:, k, h, :],
                            pt[:, idx * QT + e0 : (idx + 1) * QT],
                            start=(k == 0), stop=(k == nblk - 1),
                        )

                for pr in range(nblk // 2):
                    st = ps.tile([128, 2 * QT], F32, name="st", tag="st", bufs=2)
                    pt = ptp.tile([128, 2 * QT], F16, name="pt", tag="pt", bufs=4)
                    halves = []
                    diag = 2 * pr >= 4 * j
                    for idx in range(2):
                        k = 2 * pr + idx
                        m = k - 4 * j
                        e0 = 0 if m < 0 else m * 128
                        halves.append((idx, k, e0))
                        nc.tensor.matmul(
                            st[:, idx * QT + e0 : (idx + 1) * QT],
                            ktX[:, hb, k * 128 : (k + 1) * 128],
                            qt[:, hb, q0 + e0 : q0 + QT],
                            start=True, stop=True,
                        )
                    if not diag:
                        nc.scalar.activation(pt[:], st[:], EXP, scale=0.125)
                    else:
                        # one EXP spanning both halves' live ranges; the gap
                        # region between them is never read by the PV matmuls,
                        # and one ACT instruction saves ~300ns fixed overhead
                        e0 = halves[0][2]
                        nc.scalar.activation(
                            pt[:, e0 : 2 * QT], st[:, e0 : 2 * QT],
                            EXP, scale=0.125,
                        )
                        for idx, k, eh in halves:
                            nc.vector.tensor_mul(
                                pt[:, idx * QT + eh : idx * QT + eh + 128],
                                pt[:, idx * QT + eh : idx * QT + eh + 128],
                                tri[:],
                            )
                    if pend is not None:
                        do_pv(*pend)
                    pend = (halves, pt)
                do_pv(*pend)

                # normalize via the DMA lane-spread reciprocal ([1,512] DVE
                # reciprocal is lane-serial and ~20x slower). The chain's
                # cross-engine hops (gpsimd DMA -> DVE recip -> gpsimd DMA ->
                # gpsimd bcast -> DVE mul) are DEFERRED: fin1 is emitted one
                # head later and fin2 two heads later, so strict-FIFO engines
                # never head-of-line block on an in-flight chain.
                yu = yup.tile([D + 1, QT], F32, name="yu")
                nc.vector.tensor_copy(yu[:], ytps[:])
                s128 = npool.tile([128, 4], F32, name="s128", tag="s128")
                nc.gpsimd.dma_start(out=s128[:], in_=yu[D : D + 1, :])
                cell = {}

                def fin1():
                    r128 = npool.tile([128, 4], F32, name="r128", tag="r128")
                    nc.vector.reciprocal(r128[:], s128[:])
                    rrow = npool.tile([1, QT], F32, name="rrow", tag="rrow", bufs=4)
                    nc.gpsimd.dma_start(out=rrow[:], in_=r128[:])
                    rbc = npool.tile([D, QT], F32, name="rbc", tag="rbc", bufs=4)
                    nc.gpsimd.partition_broadcast(rbc[:], rrow[:])
                    cell["rbc"] = rbc

                def fin2():
                    nc.vector.tensor_mul(
                        yt[hr : hr + D, hb, q0 : q0 + QT], yu[0:D, :], cell["rbc"][:]
                    )

                return fin1, fin2

            op_alt = [0]

            def outproj_pieces(jo):
                pieces = []

                def op_piece(tb, co):
                    op = ps.tile([128, QT], F32, name="op", tag="pj", bufs=2)
                    for hb in range(HB):
                        nc.tensor.matmul(
                            op[:],
                            yt[:, hb, tb * 128 : (tb + 1) * 128],
                            wo_sb[:, hb, co * QT : (co + 1) * QT],
                            start=(hb == 0), stop=(hb == HB - 1),
                        )
                    ot = opool.tile([128, QT], F32, name="ot")
                    if op_alt[0] % 2 == 0:
                        nc.scalar.copy(ot[:], op[:])
                    else:
                        nc.vector.tensor_copy(ot[:], op[:])
                    op_alt[0] += 1
                    nc.sync.dma_start(
                        out=out_d.ap()[
                            tb * 128 : (tb + 1) * 128, co * QT : (co + 1) * QT
                        ],
                        in_=ot[:],
                    )

                for tb in range(4 * jo, 4 * (jo + 1)):
                    for co in range(C // QT):
                        pieces.append(lambda tb=tb, co=co: op_piece(tb, co))
                return pieces

            # ---- software-pipelined emission ----
            s1p0 = stage1_pieces(0)
            s1p1 = stage1_pieces(1)
            s1p0[0]()  # x^T transposes first (sync + scalar queues)
            load_weights()  # in parallel on gpsimd/scalar/sync
            s1p1[0]()  # prefetch tile 1's x^T too: attention(0) is too
            # small to hide a 10us serialized transpose burst
            for p in s1p0[1:]:
                p()
            for j in range(NJ):
                fill = []
                if j == 0:
                    fill.extend(s1p1[1:])
                elif j + 1 < NJ:
                    fill.extend(stage1_pieces(j + 1))
                if j > 0:
                    fill.extend(outproj_pieces(j - 1))
                # spread fill pieces across the 8 attention heads
                per = [fill[(h * len(fill)) // HPC : ((h + 1) * len(fill)) // HPC]
                       for h in range(HPC)]
                fins = []
                for h in range(HPC):
                    fins.append(attention(j, h))
                    if h >= 1:
                        fins[h - 1][0]()  # recip+bcast of previous head
                    if h >= 2:
                        fins[h - 2][1]()  # norm mul two heads back
                    for p in per[h]:
                        p()
                fins[HPC - 1][0]()
                fins[HPC - 2][1]()
                fins[HPC - 1][1]()
            for p in outproj_pieces(NJ - 1):
                p()
    nc.finalize()
    return nc


def _tables():
    # head-dim-permuted rope tables, tiled for 2 heads per 128 partitions
    freqs = 1.0 / (10000.0 ** (np.arange(0, D, 2, dtype=np.float64) / D))  # [32]
    grid = np.arange(T, dtype=np.float64)[None, :] * freqs[:, None]  # [32, T]
    cos_p, sin_p = np.cos(grid), np.sin(grid)
    cos64 = np.zeros((D, T))
    sin64 = np.zeros((D, T))
    for n in range(D):
        g, r = n // 32, n % 32
        q, i = r // 16, r % 16
        p = 16 * g + i
        cos64[n] = cos_p[p]
        sin64[n] = sin_p[p] if q == 0 else -sin_p[p]
    cosb = np.tile(cos64, (2, 1)).astype(np.float32)
    sinb = np.tile(sin64, (2, 1)).astype(np.float32)
    return np.ascontiguousarray(cosb), np.ascontiguousarray(sinb)


def _perm():
    # permuted-to-original head-dim map, replicated across all heads
    perm = np.zeros(D, dtype=np.int64)
    for n in range(D):
        g, r = n // 32, n % 32
        q, i = r // 16, r % 16
        perm[n] = 2 * (16 * g + i) + q
    return (np.arange(NH)[:, None] * D + perm[None, :]).reshape(-1)


def kernel(x, wq, wk, wv, wo):
    if "nc" not in _CACHE:
        _CACHE["nc"] = build()
    nc = _CACHE["nc"]

    cosb, sinb = _tables()
    fp = _perm()
    x16 = np.ascontiguousarray(x, dtype=np.float32).astype(np.float16)
    wq_p = wq[fp]
    wk_p = wk[fp]
    core_ids = list(range(NCORES))
    in_maps = []
    for c in core_ids:
        b, hg = divmod(c, 2)
        ch = slice(hg * HD, (hg + 1) * HD)
        in_maps.append(
            {
                "x": np.ascontiguousarray(x16[b]),
                "wqt": np.ascontiguousarray(wq_p[ch].T.astype(np.float16)),
                "wkt": np.ascontiguousarray(wk_p[ch].T.astype(np.float16)),
                "wvt": np.ascontiguousarray(wv[ch].T.astype(np.float16)),
                "wot": np.ascontiguousarray(wo[:, ch].T.astype(np.float16)),
                "cosb": cosb,
                "sinb": sinb,
            }
        )
    try:
        res = run_bass_kernel_spmd(nc, in_maps, core_ids).results
    except Exception:
        # transient NRT/device hiccup: retry once
        res = run_bass_kernel_spmd(nc, in_maps, core_ids).results
    out = np.zeros((B, T, C), dtype=np.float32)
    for c in core_ids:
        out[c // 2] += res[c]["out"]
    return out


# revision 3
# speedup vs baseline: 1.0212x; 1.0212x over previous
"""Causal self-attention (B=4, T=2048, C=1024, 16 heads, interleaved RoPE)
on 8 trn2 NeuronCores.

Sharding: hybrid batch x head-half. Core c owns batch c//2 and heads
8*(c%2)..8*(c%2)+7 (512 channels). Host sums the two partial [T, C]
outputs per batch (the all-reduce of the hinted TP scheme).

Per-core data path is fp16 (f32 PSUM accumulation), which keeps every
matmul at 1 PE-cycle/row and enables:
  - x^T tiles loaded straight from DRAM with the DMA XBAR transpose
    (zero PE transposes for x),
  - interleaved RoPE without PE matmuls: head dims are pre-permuted
    host-side so the even/odd pair swap becomes a 16-lane swap within
    32-lane groups, done by one DVE stream_shuffle,
  - V projected token-major directly (x^T chunks stationary), so V^T
    needs no transposes either,
  - exact block-causal score/PV ranges (fp16 has no N>=256 rate cliff).
Scores S^T[kv, q] = K Q^T per head in double-wide [128, 1024] PSUM
tiles, with K^T zero-padded to K=128 (ktE/ktO copies per head parity) —
K=64 matmuls measure at half PE rate, K=128 at full rate. One merged
exp per pair on ACT (scale=1/8, no max subtraction; scores ~N(0,1));
causal via column subranges + one triangular mask multiply per diagonal
block. y^T = V_aug^T @ P^T with a fused ones-column row-sum;
normalization via DMA lane-spread reciprocal + gpsimd broadcast, with
the chain's cross-engine stages deferred by 1-2 heads so strict-FIFO
engines never head-of-line block. Score->exp->PV is software-pipelined
(S of pair p+1 issues before PV of pair p), x^T of tiles 0 and 1 is
prefetched at startup across both hwdge queues, and stage-1/out-proj
pieces are interleaved between attention heads to keep the PE stream
dense.

Self-contained: hardcoded shapes, no reads of /root/problem/*.
"""
import numpy as np

import concourse.bacc as bacc
import concourse.mybir as mybir
import concourse.tile as tile
from concourse.bass_utils import run_bass_kernel_spmd
from concourse.masks import make_upper_triangular

B, T, C = 4, 2048, 1024
NH, D = 16, 64
NCORES = 8
HPC = 8  # heads per core
HD = HPC * D  # per-core head channels = 512
CB = C // 128  # input channel blocks = 8
HB = HD // 128  # head-channel blocks = 4
QT = 512
NJ = T // QT  # q tiles = 4
KB = T // 128  # kv blocks = 16
F16 = mybir.dt.float16
F32 = mybir.dt.float32
EXP = mybir.ActivationFunctionType.Exp
SWAP16 = list(range(16, 32)) + list(range(16))

_CACHE = {}


def build():
    nc = bacc.Bacc(None, target_bir_lowering=False)
    x_d = nc.declare_dram_parameter("x", [T, C], F16, isOutput=False)
    wq_d = nc.declare_dram_parameter("wqt", [C, HD], F16, isOutput=False)
    wk_d = nc.declare_dram_parameter("wkt", [C, HD], F16, isOutput=False)
    wv_d = nc.declare_dram_parameter("wvt", [C, HD], F16, isOutput=False)
    wo_d = nc.declare_dram_parameter("wot", [HD, C], F16, isOutput=False)
    cos_d = nc.declare_dram_parameter("cosb", [128, T], F32, isOutput=False)
    sin_d = nc.declare_dram_parameter("sinb", [128, T], F32, isOutput=False)
    out_d = nc.declare_dram_parameter("out", [T, C], F32, isOutput=True)

    with tile.TileContext(nc) as tc:
        with (
            tc.tile_pool(name="const", bufs=1) as const,
            tc.tile_pool(name="wpool", bufs=1) as wpool,
            tc.tile_pool(name="xtp", bufs=2) as xtp,
            tc.tile_pool(name="rope", bufs=2) as rope,
            tc.tile_pool(name="qkp", bufs=1) as qkp,
            tc.tile_pool(name="ptp", bufs=4) as ptp,
            tc.tile_pool(name="yup", bufs=4) as yup,
            tc.tile_pool(name="npool", bufs=4) as npool,
            tc.tile_pool(name="opool", bufs=4) as opool,
            tc.tile_pool(name="ps", bufs=2, space="PSUM") as ps,
        ):
            # ---- constants ----
            cos_t = const.tile([128, T], F32)
            sin_t = const.tile([128, T], F32)
            tri = const.tile([128, 128], F16)
            ones16 = const.tile([128, KB * HPC], F16)
            with tc.tile_pool(name="wstage", bufs=1) as wstage:
                tri_f = wstage.tile([128, 128], F32, tag="trif")
                make_upper_triangular(nc, tri_f, val=1.0, diag=True)  # 1 if i<=j
                nc.vector.tensor_copy(tri[:], tri_f[:])
                ones_f = wstage.tile([128, KB * HPC], F32, tag="onesf")
                nc.gpsimd.memset(ones_f[:], 1.0)
                nc.vector.tensor_copy(ones16[:], ones_f[:])

            # ---- weights (fp16, no casting needed) ----
            wq_sb = wpool.tile([128, CB, HD], F16)
            wk_sb = wpool.tile([128, CB, HD], F16)
            wv_sb = wpool.tile([128, CB, HD], F16)
            wo_sb = wpool.tile([128, HB, C], F16)

            def load_weights():
                # spread across the three DMA-capable queues so nothing
                # serializes behind the startup x^T transposes
                nc.gpsimd.dma_start(
                    out=wq_sb[:], in_=wq_d.ap().rearrange("(cb p) m -> p cb m", p=128)
                )
                nc.gpsimd.dma_start(
                    out=wv_sb[:], in_=wv_d.ap().rearrange("(cb p) m -> p cb m", p=128)
                )
                nc.gpsimd.dma_start(out=sin_t[:], in_=sin_d[:])
                nc.scalar.dma_start(
                    out=wk_sb[:], in_=wk_d.ap().rearrange("(cb p) m -> p cb m", p=128)
                )
                nc.sync.dma_start(out=cos_t[:], in_=cos_d[:])
                nc.scalar.dma_start(
                    out=wo_sb[:], in_=wo_d.ap().rearrange("(hb p) m -> p hb m", p=128)
                )

            # ---- persistent per-batch tensors ----
            # K^T stored twice, zero-padded to K=128 per head parity: even
            # heads live in rows 0-63 of ktE (rows 64-127 zero), odd heads in
            # rows 64-127 of ktO. Scores then run with K=128 (full PE rate;
            # K=64 matmuls measure at half rate), the garbage rows of the
            # moving Q operand multiply zeros.
            qt = qkp.tile([128, HB, T], F16)
            ktE = qkp.tile([128, HB, T], F16)
            ktO = qkp.tile([128, HB, T], F16)
            yt = qkp.tile([128, HB, T], F16)
            va = qkp.tile([128, KB, HPC, D + 1], F16)
            nc.vector.memset(ktE[D:128, :, :], 0.0)
            nc.vector.memset(ktO[0:D, :, :], 0.0)
            nc.vector.tensor_copy(
                va[:, :, :, D : D + 1], ones16[:].rearrange("p (k h) -> p k h", h=HPC)
            )

            xts = {}

            def stage1_pieces(tt):
                """Closures: transpose-load + project + rope 512 tokens."""
                t0 = tt * QT
                pieces = []

                def load_xt():
                    xts[tt] = xtp.tile(
                        [128, CB, QT], F16, name=f"xt_{tt}", tag="xt", bufs=3
                    )
                    for cb in range(CB):
                        # at startup scalar is idle: split transposes across
                        # both hwdge queues to halve time-to-first-matmul
                        eng = nc.scalar if (tt == 0 and cb % 2) else nc.sync
                        eng.dma_start(
                            out=xts[tt][:, cb],
                            in_=x_d.ap()[t0 : t0 + QT, cb * 128 : (cb + 1) * 128],
                            transpose=True,
                        )

                pieces.append(load_xt)

                def qk_piece(wsb, dst, hb):
                    xt = xts[tt]
                    pj = ps.tile([128, QT], F32, name="pj", tag="pj", bufs=2)
                    for cb in range(CB):
                        nc.tensor.matmul(
                            pj[:], wsb[:, cb, hb * 128 : (hb + 1) * 128], xt[:, cb],
                            start=(cb == 0), stop=(cb == CB - 1),
                        )
                    pjc = rope.tile([128, QT], F16, name="pjc", tag="pjc", bufs=2)
                    nc.vector.tensor_mul(pjc[:], pj[:], cos_t[:, t0 : t0 + QT])
                    pjs = rope.tile([128, QT], F16, name="pjs", tag="pjs", bufs=2)
                    nc.vector.tensor_mul(pjs[:], pj[:], sin_t[:, t0 : t0 + QT])
                    pjss = rope.tile([128, QT], F16, name="pjss", tag="pjss", bufs=2)
                    nc.vector.stream_shuffle(pjss[:], pjs[:], SWAP16)
                    if dst is qt:
                        nc.vector.tensor_add(dst[:, hb, t0 : t0 + QT], pjc[:], pjss[:])
                    else:
                        nc.vector.tensor_add(
                            ktE[0:D, hb, t0 : t0 + QT], pjc[0:D], pjss[0:D]
                        )
                        nc.vector.tensor_add(
                            ktO[D:128, hb, t0 : t0 + QT], pjc[D:128], pjss[D:128]
                        )

                for wsb, dst in ((wq_sb, qt), (wk_sb, None)):
                    for hb in range(HB):
                        pieces.append(
                            lambda wsb=wsb, dst=dst, hb=hb: qk_piece(wsb, dst, hb)
                        )

                def v_piece(tb):
                    xt = xts[tt]
                    pj = ps.tile([128, QT], F32, name="pjv", tag="pj", bufs=2)
                    for cb in range(CB):
                        nc.tensor.matmul(
                            pj[:], xt[:, cb, tb * 128 : (tb + 1) * 128], wv_sb[:, cb],
                            start=(cb == 0), stop=(cb == CB - 1),
                        )
                    kv = tt * 4 + tb
                    nc.vector.tensor_copy(
                        va[:, kv, :, 0:D], pj[:].rearrange("p (h d) -> p h d", d=D)
                    )

                for tb in range(4):
                    pieces.append(lambda tb=tb: v_piece(tb))
                return pieces

            def attention(j, h):
                hr = (h % 2) * D
                hb = h // 2
                ktX = ktO if (h % 2) else ktE
                q0 = j * QT
                nblk = 4 * (j + 1)
                ytps = ps.tile([D + 1, QT], F32, name="ytps", tag="yt", bufs=2)
                pend = None

                def do_pv(halves, pt):
                    for idx, k, e0 in halves:
                        nc.tensor.matmul(
                            ytps[:, e0:QT],
                            va[:, k, h, :],
                            pt[:, idx * QT + e0 : (idx + 1) * QT],
                            start=(k == 0), stop=(k == nblk - 1),
                        )

                for pr in range(nblk // 2):
                    st = ps.tile([128, 2 * QT], F32, name="st", tag="st", bufs=2)
                    pt = ptp.tile([128, 2 * QT], F16, name="pt", tag="pt", bufs=4)
                    halves = []
                    diag = 2 * pr >= 4 * j
                    for idx in range(2):
                        k = 2 * pr + idx
                        m = k - 4 * j
                        e0 = 0 if m < 0 else m * 128
                        halves.append((idx, k, e0))
                        nc.tensor.matmul(
                            st[:, idx * QT + e0 : (idx + 1) * QT],
                            ktX[:, hb, k * 128 : (k + 1) * 128],
                            qt[:, hb, q0 + e0 : q0 + QT],
                            start=True, stop=True,
                        )
                    if not diag:
                        nc.scalar.activation(pt[:], st[:], EXP, scale=0.125)
                    else:
                        # one EXP spanning both halves' live ranges; the gap
                        # region between them is never read by the PV matmuls,
                        # and one ACT instruction saves ~300ns fixed overhead
                        e0 = halves[0][2]
                        nc.scalar.activation(
                            pt[:, e0 : 2 * QT], st[:, e0 : 2 * QT],
                            EXP, scale=0.125,
                        )
                        for idx, k, eh in halves:
                            nc.vector.tensor_mul(
                                pt[:, idx * QT + eh : idx * QT + eh + 128],
                                pt[:, idx * QT + eh : idx * QT + eh + 128],
                                tri[:],
                            )
                    if pend is not None:
                        do_pv(*pend)
                    pend = (halves, pt)
                do_pv(*pend)

                # normalize via the DMA lane-spread reciprocal ([1,512] DVE
                # reciprocal is lane-serial and ~20x slower). The chain's
                # cross-engine hops (gpsimd DMA -> DVE recip -> gpsimd DMA ->
                # gpsimd bcast -> DVE mul) are DEFERRED: fin1 is emitted one
                # head later and fin2 two heads later, so strict-FIFO engines
                # never head-of-line block on an in-flight chain.
                yu = yup.tile([D + 1, QT], F32, name="yu")
                nc.vector.tensor_copy(yu[:], ytps[:])
                s128 = npool.tile([128, 4], F32, name="s128", tag="s128")
                nc.gpsimd.dma_start(out=s128[:], in_=yu[D : D + 1, :])
                cell = {}

                def fin1():
                    r128 = npool.tile([128, 4], F32, name="r128", tag="r128")
                    nc.vector.reciprocal(r128[:], s128[:])
                    rrow = npool.tile([1, QT], F32, name="rrow", tag="rrow", bufs=4)
                    nc.gpsimd.dma_start(out=rrow[:], in_=r128[:])
                    rbc = npool.tile([D, QT], F32, name="rbc", tag="rbc", bufs=4)
                    nc.gpsimd.partition_broadcast(rbc[:], rrow[:])
                    cell["rbc"] = rbc

                def fin2():
                    nc.vector.tensor_mul(
                        yt[hr : hr + D, hb, q0 : q0 + QT], yu[0:D, :], cell["rbc"][:]
                    )

                return fin1, fin2

            op_alt = [0]

            def outproj_pieces(jo):
                pieces = []

                def op_piece(tb, co):
                    op = ps.tile([128, QT], F32, name="op", tag="pj", bufs=2)
                    for hb in range(HB):
                        nc.tensor.matmul(
                            op[:],
                            yt[:, hb, tb * 128 : (tb + 1) * 128],
                            wo_sb[:, hb, co * QT : (co + 1) * QT],
                            start=(hb == 0), stop=(hb == HB - 1),
                        )
                    ot = opool.tile([128, QT], F32, name="ot")
                    if op_alt[0] % 2 == 0:
                        nc.scalar.copy(ot[:], op[:])
                    else:
                        nc.vector.tensor_copy(ot[:], op[:])
                    op_alt[0] += 1
                    nc.sync.dma_start(
                        out=out_d.ap()[
                            tb * 128 : (tb + 1) * 128, co * QT : (co + 1) * QT
                        ],
                        in_=ot[:],
                    )

                for tb in range(4 * jo, 4 * (jo + 1)):
                    for co in range(C // QT):
                        pieces.append(lambda tb=tb, co=co: op_piece(tb, co))
                return pieces

            # ---- software-pipelined emission ----
            s1p0 = stage1_pieces(0)
            s1p1 = stage1_pieces(1)
            s1p0[0]()  # x^T transposes first (sync + scalar queues)
            load_weights()  # in parallel on gpsimd/scalar/sync
            s1p1[0]()  # prefetch tile 1's x^T too: attention(0) is too
            # small to hide a 10us serialized transpose burst
            for p in s1p0[1:]:
                p()
            for j in range(NJ):
                fill = []
                if j == 0:
                    fill.extend(s1p1[1:])
                elif j + 1 < NJ:
                    fill.extend(stage1_pieces(j + 1))
                if j > 0:
                    fill.extend(outproj_pieces(j - 1))
                # spread fill pieces across the 8 attention heads
                per = [fill[(h * len(fill)) // HPC : ((h + 1) * len(fill)) // HPC]
                       for h in range(HPC)]
                fins = []
                for h in range(HPC):
                    fins.append(attention(j, h))
                    if h >= 1:
                        fins[h - 1][0]()  # recip+bcast of previous head
                    if h >= 2:
                        fins[h - 2][1]()  # norm mul two heads back
                    for p in per[h]:
                        p()
                fins[HPC - 1][0]()
                fins[HPC - 2][1]()
                fins[HPC - 1][1]()
            for p in outproj_pieces(NJ - 1):
                p()
    nc.finalize()
    return nc


def _tables():
    # head-dim-permuted rope tables, tiled for 2 heads per 128 partitions
    freqs = 1.0 / (10000.0 ** (np.arange(0, D, 2, dtype=np.float64) / D))  # [32]
    grid = np.arange(T, dtype=np.float64)[None, :] * freqs[:, None]  # [32, T]
    cos_p, sin_p = np.cos(grid), np.sin(grid)
    cos64 = np.zeros((D, T))
    sin64 = np.zeros((D, T))
    for n in range(D):
        g, r = n // 32, n % 32
        q, i = r // 16, r % 16
        p = 16 * g + i
        cos64[n] = cos_p[p]
        sin64[n] = sin_p[p] if q == 0 else -sin_p[p]
    cosb = np.tile(cos64, (2, 1)).astype(np.float32)
    sinb = np.tile(sin64, (2, 1)).astype(np.float32)
    return np.ascontiguousarray(cosb), np.ascontiguousarray(sinb)


def _perm():
    # permuted-to-original head-dim map, replicated across all heads
    perm = np.zeros(D, dtype=np.int64)
    for n in range(D):
        g, r = n // 32, n % 32
        q, i = r // 16, r % 16
        perm[n] = 2 * (16 * g + i) + q
    return (np.arange(NH)[:, None] * D + perm[None, :]).reshape(-1)


def kernel(x, wq, wk, wv, wo):
    if "nc" not in _CACHE:
        _CACHE["nc"] = build()
    nc = _CACHE["nc"]

    cosb, sinb = _tables()
    fp = _perm()
    x16 = np.ascontiguousarray(x, dtype=np.float32).astype(np.float16)
    wq_p = wq[fp]
    wk_p = wk[fp]
    core_ids = list(range(NCORES))
    in_maps = []
    for c in core_ids:
        b, hg = divmod(c, 2)
        ch = slice(hg * HD, (hg + 1) * HD)
        in_maps.append(
            {
                "x": np.ascontiguousarray(x16[b]),
                "wqt": np.ascontiguousarray(wq_p[ch].T.astype(np.float16)),
                "wkt": np.ascontiguousarray(wk_p[ch].T.astype(np.float16)),
                "wvt": np.ascontiguousarray(wv[ch].T.astype(np.float16)),
                "wot": np.ascontiguousarray(wo[:, ch].T.astype(np.float16)),
                "cosb": cosb,
                "sinb": sinb,
            }
        )
    try:
        res = run_bass_kernel_spmd(nc, in_maps, core_ids).results
    except Exception:
        # transient NRT/device hiccup: retry once
        res = run_bass_kernel_spmd(nc, in_maps, core_ids).results
    out = np.zeros((B, T, C), dtype=np.float32)
    for c in core_ids:
        out[c // 2] += res[c]["out"]
    return out


# revision 4
# speedup vs baseline: 1.0279x; 1.0065x over previous
"""Causal self-attention (B=4, T=2048, C=1024, 16 heads, interleaved RoPE)
on 8 trn2 NeuronCores.

Sharding: hybrid batch x head-half. Core c owns batch c//2 and heads
8*(c%2)..8*(c%2)+7 (512 channels). Host sums the two partial [T, C]
outputs per batch (the all-reduce of the hinted TP scheme).

Per-core data path is fp16 (f32 PSUM accumulation), which keeps every
matmul at 1 PE-cycle/row and enables:
  - x^T tiles loaded straight from DRAM with the DMA XBAR transpose
    (zero PE transposes for x),
  - interleaved RoPE without PE matmuls: head dims are pre-permuted
    host-side so the even/odd pair swap becomes a 16-lane swap within
    32-lane groups, done by one DVE stream_shuffle,
  - V projected token-major directly (x^T chunks stationary), so V^T
    needs no transposes either,
  - exact block-causal score/PV ranges (fp16 has no N>=256 rate cliff).
Scores S^T[kv, q] = K Q^T per head in double-wide [128, 1024] PSUM
tiles; exp on ACT (scale=1/8, no max subtraction; scores are ~N(0,1));
causal via column subranges + one triangular mask multiply per diagonal
block. y^T = V_aug^T @ P^T with a fused ones-column row-sum;
normalization via DMA lane-spread reciprocal + gpsimd broadcast.
Score->exp->PV is software-pipelined (S of pair p+1 issues before PV of
pair p) and stage-1/out-proj pieces are interleaved between attention
heads to keep the PE stream dense.

Self-contained: hardcoded shapes, no reads of /root/problem/*.
"""
import numpy as np

import concourse.bacc as bacc
import concourse.mybir as mybir
import concourse.tile as tile
from concourse.bass_utils import run_bass_kernel_spmd
from concourse.masks import make_upper_triangular

B, T, C = 4, 2048, 1024
NH, D = 16, 64
NCORES = 8
HPC = 8  # heads per core
HD = HPC * D  # per-core head channels = 512
CB = C // 128  # input channel blocks = 8
HB = HD // 128  # head-channel blocks = 4
QT = 512
NJ = T // QT  # q tiles = 4
KB = T // 128  # kv blocks = 16
F16 = mybir.dt.float16
F32 = mybir.dt.float32
EXP = mybir.ActivationFunctionType.Exp
SWAP16 = list(range(16, 32)) + list(range(16))

_CACHE = {}


def build():
    nc = bacc.Bacc(None, target_bir_lowering=False)
    x_d = nc.declare_dram_parameter("x", [T, C], F16, isOutput=False)
    wq_d = nc.declare_dram_parameter("wqt", [C, HD], F16, isOutput=False)
    wk_d = nc.declare_dram_parameter("wkt", [C, HD], F16, isOutput=False)
    wv_d = nc.declare_dram_parameter("wvt", [C, HD], F16, isOutput=False)
    wo_d = nc.declare_dram_parameter("wot", [HD, C], F16, isOutput=False)
    cos_d = nc.declare_dram_parameter("cosb", [128, T], F32, isOutput=False)
    sin_d = nc.declare_dram_parameter("sinb", [128, T], F32, isOutput=False)
    out_d = nc.declare_dram_parameter("out", [T, C], F32, isOutput=True)

    with tile.TileContext(nc) as tc:
        with (
            tc.tile_pool(name="const", bufs=1) as const,
            tc.tile_pool(name="wpool", bufs=1) as wpool,
            tc.tile_pool(name="xtp", bufs=2) as xtp,
            tc.tile_pool(name="rope", bufs=2) as rope,
            tc.tile_pool(name="qkp", bufs=1) as qkp,
            tc.tile_pool(name="ptp", bufs=4) as ptp,
            tc.tile_pool(name="yup", bufs=4) as yup,
            tc.tile_pool(name="npool", bufs=4) as npool,
            tc.tile_pool(name="opool", bufs=3) as opool,
            tc.tile_pool(name="ps", bufs=2, space="PSUM") as ps,
        ):
            # ---- constants ----
            cos_t = const.tile([128, T], F32)
            sin_t = const.tile([128, T], F32)
            tri = const.tile([128, 128], F16)
            ones16 = const.tile([128, KB * HPC], F16)
            with tc.tile_pool(name="wstage", bufs=1) as wstage:
                tri_f = wstage.tile([128, 128], F32, tag="trif")
                make_upper_triangular(nc, tri_f, val=1.0, diag=True)  # 1 if i<=j
                nc.vector.tensor_copy(tri[:], tri_f[:])
                ones_f = wstage.tile([128, KB * HPC], F32, tag="onesf")
                nc.gpsimd.memset(ones_f[:], 1.0)
                nc.vector.tensor_copy(ones16[:], ones_f[:])

            # ---- weights (fp16, no casting needed) ----
            wq_sb = wpool.tile([128, CB, HD], F16)
            wk_sb = wpool.tile([128, CB, HD], F16)
            wv_sb = wpool.tile([128, CB, HD], F16)
            wo_sb = wpool.tile([128, HB, C], F16)

            def load_weights():
                # spread across the three DMA-capable queues so nothing
                # serializes behind the startup x^T transposes
                nc.gpsimd.dma_start(
                    out=wq_sb[:], in_=wq_d.ap().rearrange("(cb p) m -> p cb m", p=128)
                )
                nc.gpsimd.dma_start(
                    out=wv_sb[:], in_=wv_d.ap().rearrange("(cb p) m -> p cb m", p=128)
                )
                nc.gpsimd.dma_start(out=sin_t[:], in_=sin_d[:])
                nc.scalar.dma_start(
                    out=wk_sb[:], in_=wk_d.ap().rearrange("(cb p) m -> p cb m", p=128)
                )
                nc.sync.dma_start(out=cos_t[:], in_=cos_d[:])
                nc.scalar.dma_start(
                    out=wo_sb[:], in_=wo_d.ap().rearrange("(hb p) m -> p hb m", p=128)
                )

            # ---- persistent per-batch tensors ----
            # K^T stored twice, zero-padded to K=128 per head parity: even
            # heads live in rows 0-63 of ktE (rows 64-127 zero), odd heads in
            # rows 64-127 of ktO. Scores then run with K=128 (full PE rate;
            # K=64 matmuls measure at half rate), the garbage rows of the
            # moving Q operand multiply zeros.
            qt = qkp.tile([128, HB, T], F16)
            ktE = qkp.tile([128, HB, T], F16)
            ktO = qkp.tile([128, HB, T], F16)
            yt = qkp.tile([128, HB, T], F16)
            va = qkp.tile([128, KB, HPC, D + 1], F16)
            nc.vector.memset(ktE[D:128, :, :], 0.0)
            nc.vector.memset(ktO[0:D, :, :], 0.0)
            nc.vector.tensor_copy(
                va[:, :, :, D : D + 1], ones16[:].rearrange("p (k h) -> p k h", h=HPC)
            )

            xts = {}

            def stage1_pieces(tt):
                """Closures: transpose-load + project + rope 512 tokens."""
                t0 = tt * QT
                pieces = []

                def load_xt():
                    xts[tt] = xtp.tile(
                        [128, CB, QT], F16, name=f"xt_{tt}", tag="xt", bufs=4
                    )
                    for cb in range(CB):
                        # at startup scalar is idle: split transposes across
                        # both hwdge queues to halve time-to-first-matmul
                        eng = nc.scalar if (tt == 0 and cb % 2) else nc.sync
                        eng.dma_start(
                            out=xts[tt][:, cb],
                            in_=x_d.ap()[t0 : t0 + QT, cb * 128 : (cb + 1) * 128],
                            transpose=True,
                        )

                pieces.append(load_xt)

                def qk_piece(wsb, dst, hb):
                    xt = xts[tt]
                    pj = ps.tile([128, QT], F32, name="pj", tag="pj", bufs=2)
                    for cb in range(CB):
                        nc.tensor.matmul(
                            pj[:], wsb[:, cb, hb * 128 : (hb + 1) * 128], xt[:, cb],
                            start=(cb == 0), stop=(cb == CB - 1),
                        )
                    pjc = rope.tile([128, QT], F16, name="pjc", tag="pjc", bufs=2)
                    nc.vector.tensor_mul(pjc[:], pj[:], cos_t[:, t0 : t0 + QT])
                    pjs = rope.tile([128, QT], F16, name="pjs", tag="pjs", bufs=2)
                    nc.vector.tensor_mul(pjs[:], pj[:], sin_t[:, t0 : t0 + QT])
                    pjss = rope.tile([128, QT], F16, name="pjss", tag="pjss", bufs=2)
                    nc.vector.stream_shuffle(pjss[:], pjs[:], SWAP16)
                    if dst is qt:
                        nc.vector.tensor_add(dst[:, hb, t0 : t0 + QT], pjc[:], pjss[:])
                    else:
                        nc.vector.tensor_add(
                            ktE[0:D, hb, t0 : t0 + QT], pjc[0:D], pjss[0:D]
                        )
                        nc.vector.tensor_add(
                            ktO[D:128, hb, t0 : t0 + QT], pjc[D:128], pjss[D:128]
                        )

                for wsb, dst in ((wq_sb, qt), (wk_sb, None)):
                    for hb in range(HB):
                        pieces.append(
                            lambda wsb=wsb, dst=dst, hb=hb: qk_piece(wsb, dst, hb)
                        )

                def v_piece(tb):
                    xt = xts[tt]
                    pj = ps.tile([128, QT], F32, name="pjv", tag="pj", bufs=2)
                    for cb in range(CB):
                        nc.tensor.matmul(
                            pj[:], xt[:, cb, tb * 128 : (tb + 1) * 128], wv_sb[:, cb],
                            start=(cb == 0), stop=(cb == CB - 1),
                        )
                    kv = tt * 4 + tb
                    nc.vector.tensor_copy(
                        va[:, kv, :, 0:D], pj[:].rearrange("p (h d) -> p h d", d=D)
                    )

                for tb in range(4):
                    pieces.append(lambda tb=tb: v_piece(tb))
                return pieces

            def attention(j, h):
                hr = (h % 2) * D
                hb = h // 2
                ktX = ktO if (h % 2) else ktE
                q0 = j * QT
                nblk = 4 * (j + 1)
                ytps = ps.tile([D + 1, QT], F32, name="ytps", tag="yt", bufs=2)
                pend = None

                def do_pv(halves, pt):
                    for idx, k, e0 in halves:
                        nc.tensor.matmul(
                            ytps[:, e0:QT],
                            va[:, k, h, :],
                            pt[:, idx * QT + e0 : (idx + 1) * QT],
                            start=(k == 0), stop=(k == nblk - 1),
                        )

                for pr in range(nblk // 2):
                    st = ps.tile([128, 2 * QT], F32, name="st", tag="st", bufs=2)
                    pt = ptp.tile([128, 2 * QT], F16, name="pt", tag="pt", bufs=4)
                    halves = []
                    diag = 2 * pr >= 4 * j
                    for idx in range(2):
                        k = 2 * pr + idx
                        m = k - 4 * j
                        e0 = 0 if m < 0 else m * 128
                        halves.append((idx, k, e0))
                        nc.tensor.matmul(
                            st[:, idx * QT + e0 : (idx + 1) * QT],
                            ktX[:, hb, k * 128 : (k + 1) * 128],
                            qt[:, hb, q0 + e0 : q0 + QT],
                            start=True, stop=True,
                        )
                    if not diag:
                        nc.scalar.activation(pt[:], st[:], EXP, scale=0.125)
                    else:
                        # one EXP spanning both halves' live ranges; the gap
                        # region between them is never read by the PV matmuls,
                        # and one ACT instruction saves ~300ns fixed overhead
                        e0 = halves[0][2]
                        nc.scalar.activation(
                            pt[:, e0 : 2 * QT], st[:, e0 : 2 * QT],
                            EXP, scale=0.125,
                        )
                        for idx, k, eh in halves:
                            nc.vector.tensor_mul(
                                pt[:, idx * QT + eh : idx * QT + eh + 128],
                                pt[:, idx * QT + eh : idx * QT + eh + 128],
                                tri[:],
                            )
                    if pend is not None:
                        do_pv(*pend)
                    pend = (halves, pt)
                do_pv(*pend)

                # normalize via the DMA lane-spread reciprocal ([1,512] DVE
                # reciprocal is lane-serial and ~20x slower). The chain's
                # cross-engine hops (gpsimd DMA -> DVE recip -> gpsimd DMA ->
                # gpsimd bcast -> DVE mul) are DEFERRED: fin1 is emitted one
                # head later and fin2 two heads later, so strict-FIFO engines
                # never head-of-line block on an in-flight chain.
                yu = yup.tile([D + 1, QT], F32, name="yu")
                nc.vector.tensor_copy(yu[:], ytps[:])
                s128 = npool.tile([128, 4], F32, name="s128", tag="s128")
                nc.gpsimd.dma_start(out=s128[:], in_=yu[D : D + 1, :])
                cell = {}

                def fin1():
                    r128 = npool.tile([128, 4], F32, name="r128", tag="r128")
                    nc.vector.reciprocal(r128[:], s128[:])
                    rrow = npool.tile([1, QT], F32, name="rrow", tag="rrow", bufs=4)
                    nc.gpsimd.dma_start(out=rrow[:], in_=r128[:])
                    rbc = npool.tile([D, QT], F32, name="rbc", tag="rbc", bufs=4)
                    nc.gpsimd.partition_broadcast(rbc[:], rrow[:])
                    cell["rbc"] = rbc

                def fin2():
                    nc.vector.tensor_mul(
                        yt[hr : hr + D, hb, q0 : q0 + QT], yu[0:D, :], cell["rbc"][:]
                    )

                return fin1, fin2

            op_alt = [0]

            def outproj_pieces(jo):
                pieces = []

                def op_piece(tb, co):
                    op = ps.tile([128, QT], F32, name="op", tag="pj", bufs=2)
                    for hb in range(HB):
                        nc.tensor.matmul(
                            op[:],
                            yt[:, hb, tb * 128 : (tb + 1) * 128],
                            wo_sb[:, hb, co * QT : (co + 1) * QT],
                            start=(hb == 0), stop=(hb == HB - 1),
                        )
                    ot = opool.tile([128, QT], F32, name="ot")
                    if op_alt[0] % 2 == 0:
                        nc.scalar.copy(ot[:], op[:])
                    else:
                        nc.vector.tensor_copy(ot[:], op[:])
                    op_alt[0] += 1
                    nc.sync.dma_start(
                        out=out_d.ap()[
                            tb * 128 : (tb + 1) * 128, co * QT : (co + 1) * QT
                        ],
                        in_=ot[:],
                    )

                for tb in range(4 * jo, 4 * (jo + 1)):
                    for co in range(C // QT):
                        pieces.append(lambda tb=tb, co=co: op_piece(tb, co))
                return pieces

            # ---- software-pipelined emission ----
            s1p0 = stage1_pieces(0)
            s1p1 = stage1_pieces(1)
            s1p2 = stage1_pieces(2)
            s1p3 = stage1_pieces(3)
            s1p0[0]()  # x^T transposes first (sync + scalar queues)
            load_weights()  # in parallel on gpsimd/scalar/sync
            s1p1[0]()  # prefetch tile 1's x^T too: attention(0) is too
            # small to hide a 10us serialized transpose burst
            for p in s1p0[1:]:
                p()
            # prefetch the remaining x^T tiles now: the sync queue is idle
            # from here on (only out-proj DMAs), so no transpose bursts
            # compete with the norm-chain DMAs mid-stream
            s1p2[0]()
            s1p3[0]()
            for j in range(NJ):
                fill = []
                if j == 0:
                    fill.extend(s1p1[1:])
                elif j + 1 < NJ:
                    fill.extend([s1p2, s1p3][j - 1][1:])
                if j > 0:
                    fill.extend(outproj_pieces(j - 1))
                # spread fill pieces across the 8 attention heads
                per = [fill[(h * len(fill)) // HPC : ((h + 1) * len(fill)) // HPC]
                       for h in range(HPC)]
                fins = []
                for h in range(HPC):
                    fins.append(attention(j, h))
                    if h >= 1:
                        fins[h - 1][0]()  # recip+bcast of previous head
                    if h >= 2:
                        fins[h - 2][1]()  # norm mul two heads back
                    for p in per[h]:
                        p()
                fins[HPC - 1][0]()
                fins[HPC - 2][1]()
                fins[HPC - 1][1]()
            for p in outproj_pieces(NJ - 1):
                p()
    nc.finalize()
    return nc


def _tables():
    # head-dim-permuted rope tables, tiled for 2 heads per 128 partitions
    freqs = 1.0 / (10000.0 ** (np.arange(0, D, 2, dtype=np.float64) / D))  # [32]
    grid = np.arange(T, dtype=np.float64)[None, :] * freqs[:, None]  # [32, T]
    cos_p, sin_p = np.cos(grid), np.sin(grid)
    cos64 = np.zeros((D, T))
    sin64 = np.zeros((D, T))
    for n in range(D):
        g, r = n // 32, n % 32
        q, i = r // 16, r % 16
        p = 16 * g + i
        cos64[n] = cos_p[p]
        sin64[n] = sin_p[p] if q == 0 else -sin_p[p]
    cosb = np.tile(cos64, (2, 1)).astype(np.float32)
    sinb = np.tile(sin64, (2, 1)).astype(np.float32)
    return np.ascontiguousarray(cosb), np.ascontiguousarray(sinb)


def _perm():
    # permuted-to-original head-dim map, replicated across all heads
    perm = np.zeros(D, dtype=np.int64)
    for n in range(D):
        g, r = n // 32, n % 32
        q, i = r // 16, r % 16
        perm[n] = 2 * (16 * g + i) + q
    return (np.arange(NH)[:, None] * D + perm[None, :]).reshape(-1)


def kernel(x, wq, wk, wv, wo):
    if "nc" not in _CACHE:
        _CACHE["nc"] = build()
    nc = _CACHE["nc"]

    cosb, sinb = _tables()
    fp = _perm()
    x16 = np.ascontiguousarray(x, dtype=np.float32).astype(np.float16)
    wq_p = wq[fp]
    wk_p = wk[fp]
    core_ids = list(range(NCORES))
    in_maps = []
    for c in core_ids:
        b, hg = divmod(c, 2)
        ch = slice(hg * HD, (hg + 1) * HD)
        in_maps.append(
            {
                "x": np.ascontiguousarray(x16[b]),
                "wqt": np.ascontiguousarray(wq_p[ch].T.astype(np.float16)),
                "wkt": np.ascontiguousarray(wk_p[ch].T.astype(np.float16)),
                "wvt": np.ascontiguousarray(wv[ch].T.astype(np.float16)),
                "wot": np.ascontiguousarray(wo[:, ch].T.astype(np.float16)),
                "cosb": cosb,
                "sinb": sinb,
            }
        )
    try:
        res = run_bass_kernel_spmd(nc, in_maps, core_ids).results
    except Exception:
        # transient NRT/device hiccup: retry once
        res = run_bass_kernel_spmd(nc, in_maps, core_ids).results
    out = np.zeros((B, T, C), dtype=np.float32)
    for c in core_ids:
        out[c // 2] += res[c]["out"]
    return out
